# revision 11
# baseline (speedup 1.0000x reference)
"""HSMM generative forward — Bass kernel for 8 TRN2 NeuronCores.

Data-parallel over batch: each core handles 128 of the 1024 examples
end-to-end; the (tiny) transition factors and the (large) MLP weights are
replicated. No collectives.

Per-core pipeline (B=128 examples, K=128 states, T=60 steps, L=6 lags):

  Phase A (feed-forward, PE + DMA bound):
    pi    = softmax(u @ W_init)              -> u_ring[0] (prob space)
    hT    = relu(W_c1^T @ u^T)               [8192+1 x B] bf16, transposed
    condT = (h @ W_c2)^T                     [16384 x B] bf16, transposed
            (streamed in 512-col chunks; psum -> bf16 -> PE-transpose)
    per example b: trans_raw = tsc + cf_b @ ct_b^T  (PE, K=64)
                   P_b = softmax_rows(trans_raw)    [k x j] bf16

  Phase B (recurrence, prob space — no per-step logsumexp):
    a_t[k,b]  = sum_l u_{t-1-l}[k,b] * exp(obs[t,l,k,b] + len_lp)
    astar_t[j,b] = sum_k P_b[k,j] a_t[k,b]  via per-example PE matvec
                (lhsT = P_b [k,j], rhs = a_t column -> psum column b)
    running per-example scale C[b], renormalized via ones-matmul colsum
    out[b]    = C[b] + log(sum_k a_59[k,b])

All biases are folded into an extra weight row (host appends a ones row to
activations). Host pre-tiles/pre-transposes/pre-casts inputs so every DMA
is contiguous per partition.
"""

import numpy as np
import ml_dtypes

BF16 = ml_dtypes.bfloat16

K = 128
A_DIM = 64
L = 6
T = 60
BSZ = 1024
TH2 = 1024
B = 128           # batch shard per core
N_CORES = 8
NEGINF = -1e38
LEN_LP = float(-np.log(L))

I1_TILES = 9      # ceil((1024+1)/128): uniqenc + bias row
I2_TILES = 65     # ceil((8192+1)/128): h + bias row
C2 = K * 2 * A_DIM            # 16384 cond cols
CCHUNK = 512
N_CHUNKS = C2 // CCHUNK       # 32
I2_GROUPS = [17, 17, 17, 14]  # i2-tile groups for W_c2 staging
RENORM_EVERY = 4

_CACHE = {}
_WSPLIT_UID = [0]


def _split_excess_waits(nc, max_waits=1):
    """This walrus build rejects instructions whose sync_info.on_wait has
    more than `max_waits` entries ("Too many sync wait commands").  Hoist
    excess waits onto preceding same-engine NoOps — the engine executes in
    order, so waiting on a preceding NoOp is semantically identical."""
    from concourse import mybir

    n_split = 0
    for fn in nc.m.functions:
        for blk in fn.blocks:
            insts = blk.instructions
            new_insts = []
            changed = False
            for inst in insts:
                si = getattr(inst, "sync_info", None)
                waits = list(si.on_wait) if si is not None else []
                if len(waits) > max_waits:
                    changed = True
                    n_split += 1
                    extra = waits[:-max_waits]
                    keep = waits[len(extra):]
                    for i in range(0, len(extra), max_waits):
                        chunk = extra[i:i + max_waits]
                        _WSPLIT_UID[0] += 1
                        nop = mybir.InstNoOp(
                            name=f"I-wsplit-{_WSPLIT_UID[0]}", ins=[], outs=[])
                        nop.engine = inst.engine
                        nop.sync_info = mybir.SyncInfo(on_wait=chunk,
                                                       on_update=[])
                        new_insts.append(nop)
                    si.on_wait = keep
                new_insts.append(inst)
            if changed:
                blk.instructions = new_insts
    return n_split


def _stage0_pi(nc, tc, bass, mybir, AF, uT_sb, Wi_sb, smallp, ident_f32,
               u_ring):
    f32 = mybir.dt.float32
    with tc.tile_pool(name="ps0", bufs=2, space=bass.MemorySpace.PSUM) as ps0:
        ps_z = ps0.tile([128, 128], f32, tag="ps_z")
        for i in range(I1_TILES):
            nc.tensor.matmul(ps_z[:], uT_sb[:, i, :], Wi_sb[:, i, :],
                             start=(i == 0), stop=(i == I1_TILES - 1))
        mx = smallp.tile([128, 1], f32, tag="mx")
        nc.vector.reduce_max(out=mx[:], in_=ps_z[:], axis=mybir.AxisListType.X)
        negm = smallp.tile([128, 1], f32, tag="negm")
        nc.vector.tensor_scalar_mul(negm[:], mx[:], -1.0)
        esum = smallp.tile([128, 1], f32, tag="esum")
        e_bk = smallp.tile([128, 128], f32, tag="e_bk")
        nc.scalar.activation(e_bk[:], ps_z[:], AF.Exp, bias=negm[:],
                             scale=1.0, accum_out=esum[:])
        rz = smallp.tile([128, 1], f32, tag="rz")
        nc.vector.reciprocal(rz[:], esum[:])
        u0_bk = smallp.tile([128, 128], f32, tag="u0_bk")
        nc.vector.tensor_scalar_mul(u0_bk[:], e_bk[:], rz[:])
        ps_t0 = ps0.tile([128, 128], f32, tag="ps_z")
        nc.tensor.transpose(ps_t0[:], u0_bk[:], ident_f32[:])
        nc.vector.tensor_copy(u_ring[0][:], ps_t0[:])


def _stage1_hT(nc, tc, bass, mybir, AF, Wc1, uT_sb, hT_sb):
    f32 = mybir.dt.float32
    bf16 = mybir.dt.bfloat16
    with (
        tc.tile_pool(name="w1stage", bufs=3) as w1p,
        tc.tile_pool(name="psH", bufs=2, space=bass.MemorySpace.PSUM) as psH,
    ):
        for ci in range(64):
            w1t = w1p.tile([128, I1_TILES, 128], bf16, tag="w1t")
            nc.sync.dma_start(w1t[:], Wc1[ci])
            ps_h = psH.tile([128, 128], f32, tag="ps_h")
            for i in range(I1_TILES):
                nc.tensor.matmul(ps_h[:], w1t[:, i, :], uT_sb[:, i, :],
                                 start=(i == 0), stop=(i == I1_TILES - 1))
            nc.scalar.activation(hT_sb[:, ci, :], ps_h[:], AF.Relu)
    # bias row tile (i2-tile 64): row0 = 1, rest 0
    nc.vector.memset(hT_sb[:, 64, :], 0.0)
    nc.vector.memset(hT_sb[0:1, 64, :], 1.0)


def _stage2_condT(nc, tc, bass, mybir, Wc2, hT_sb, cfT_sb, ctT_sb, ident_bf):
    f32 = mybir.dt.float32
    bf16 = mybir.dt.bfloat16
    with (
        tc.tile_pool(name="w2stage", bufs=3) as w2p,
        tc.tile_pool(name="ctmp", bufs=4) as ctmpp,
        tc.tile_pool(name="psC", bufs=2, space=bass.MemorySpace.PSUM) as psC,
        tc.tile_pool(name="psT", bufs=4, space=bass.MemorySpace.PSUM) as psT,
    ):
        for ci in range(N_CHUNKS):
            ps_c = psC.tile([128, CCHUNK], f32, tag="ps_c")
            g0 = 0
            ii = 0
            for gl in I2_GROUPS:
                w2t = w2p.tile([128, I2_GROUPS[0], CCHUNK], bf16, tag="w2t")
                nc.sync.dma_start(w2t[:, 0:gl, :], Wc2[ci, :, g0:g0 + gl, :])
                for i in range(gl):
                    nc.tensor.matmul(ps_c[:], hT_sb[:, g0 + i, :], w2t[:, i, :],
                                     start=(ii == 0), stop=(ii == I2_TILES - 1))
                    ii += 1
                g0 += gl
            for sub in range(CCHUNK // 128):
                kt = ci * 4 + sub
                for half, dst in ((0, cfT_sb), (1, ctT_sb)):
                    c0 = 128 * sub + 64 * half
                    cbf = ctmpp.tile([128, 64], bf16, tag="cbf")
                    nc.vector.tensor_copy(cbf[:], ps_c[:, c0:c0 + 64])
                    ps_tr = psT.tile([64, 128], bf16, tag="ps_tr")
                    nc.tensor.transpose(ps_tr[:], cbf[:], ident_bf[:])
                    nc.vector.tensor_copy(dst[:, kt, :], ps_tr[:])


def _stage3_P(nc, tc, bass, mybir, AF, cfT_sb, ctT_sb, tsc_sb, P_sb):
    f32 = mybir.dt.float32
    with (
        tc.tile_pool(name="ptmp", bufs=3) as ptmpp,
        tc.tile_pool(name="psB", bufs=3, space=bass.MemorySpace.PSUM) as psB,
    ):
        for b in range(B):
            ps_tr = psB.tile([K, K], f32, tag="ps_b")
            nc.tensor.matmul(ps_tr[:], cfT_sb[:, :, b],
                             ctT_sb[:, :, b], start=True, stop=True)
            tr = ptmpp.tile([K, K], f32, tag="tr")
            nc.vector.tensor_add(tr[:], ps_tr[:], tsc_sb[:])
            mxb = ptmpp.tile([K, 1], f32, tag="mxb")
            nc.vector.reduce_max(out=mxb[:], in_=tr[:],
                                 axis=mybir.AxisListType.X)
            negmb = ptmpp.tile([K, 1], f32, tag="negmb")
            nc.vector.tensor_scalar_mul(negmb[:], mxb[:], -1.0)
            zsb = ptmpp.tile([K, 1], f32, tag="zsb")
            eb = ptmpp.tile([K, K], f32, tag="eb")
            nc.scalar.activation(eb[:], tr[:], AF.Exp, bias=negmb[:],
                                 scale=1.0, accum_out=zsb[:])
            rzb = ptmpp.tile([K, 1], f32, tag="rzb")
            nc.vector.reciprocal(rzb[:], zsb[:])
            nc.vector.tensor_scalar_mul(P_sb[:, b, :], eb[:], rzb[:])


def _recurrence(nc, tc, bass, mybir, AF, P_sb, obsT, u_ring, Cacc, ones_col,
                ones_row, lenlp_sb, out):
    f32 = mybir.dt.float32
    bf16 = mybir.dt.bfloat16
    with (
        tc.tile_pool(name="obs", bufs=6) as obsp,
        tc.tile_pool(name="rec", bufs=2) as recp,
        tc.tile_pool(name="psR", bufs=2, space=bass.MemorySpace.PSUM) as psR,
        tc.tile_pool(name="psV", bufs=2, space=bass.MemorySpace.PSUM) as psV,
        tc.tile_pool(name="psW", bufs=2, space=bass.MemorySpace.PSUM) as psW,
    ):
        for t in range(T):
            obs_t = obsp.tile([128, L, B], bf16, tag="obs_t")
            nc.sync.dma_start(obs_t[:], obsT[t])
            e_t = recp.tile([128, L, B], f32, tag="e_t")
            nc.scalar.activation(e_t[:], obs_t[:], AF.Exp, bias=lenlp_sb[:],
                                 scale=1.0)

            # a_t = sum_l u_{t-1-l} * e_l ; lag l lives in ring[(t-l) % 7]
            lags = [l for l in range(L) if l <= t]
            a_t = recp.tile([128, B], bf16, tag="a_t")
            pr0 = recp.tile([128, B], f32, tag="pr0")
            pr1 = recp.tile([128, B], f32, tag="pr1")
            pr2 = recp.tile([128, B], f32, tag="pr2")
            # products for lags >= 1 first (off the critical path), then lag 0
            acc = None
            scratch = pr2
            for l in lags[1:][::-1]:
                pr = pr0 if acc is None else pr1
                nc.vector.tensor_mul(pr[:], u_ring[(t - l) % 7][:],
                                     e_t[:, l, :])
                if acc is None:
                    acc = pr
                else:
                    nc.vector.tensor_add(scratch[:], acc[:], pr[:])
                    acc, scratch = scratch, acc
            m0 = pr1
            nc.vector.tensor_mul(m0[:], u_ring[t % 7][:], e_t[:, 0, :])
            if acc is None:
                nc.vector.tensor_copy(a_t[:], m0[:])
            else:
                nc.vector.tensor_add(a_t[:], acc[:], m0[:])

            if t == T - 1:
                ps_s = psV.tile([1, B], f32, tag="ps_s")
                nc.tensor.matmul(ps_s[:], ones_col[:], a_t[:],
                                 start=True, stop=True)
                lg = recp.tile([1, B], f32, tag="lg")
                nc.scalar.activation(lg[:], ps_s[:], AF.Ln)
                outv = recp.tile([1, B], f32, tag="outv")
                nc.vector.tensor_add(outv[:], lg[:], Cacc[:])
                nc.sync.dma_start(out[:], outv[:])
                break

            # astar: per-example matvec, psum column b
            ps_as = psR.tile([128, B], f32, tag="ps_as")
            for b in range(B):
                nc.tensor.matmul(ps_as[:, b:b + 1], P_sb[:, b, :],
                                 a_t[:, b:b + 1], start=True, stop=True)

            u_new = u_ring[(t + 1) % 7]
            if t % RENORM_EVERY == RENORM_EVERY - 1:
                ps_s = psV.tile([1, B], f32, tag="ps_s")
                nc.tensor.matmul(ps_s[:], ones_col[:], a_t[:],
                                 start=True, stop=True)
                lg = recp.tile([1, B], f32, tag="lg")
                nc.scalar.activation(lg[:], ps_s[:], AF.Ln)
                nc.vector.tensor_add(Cacc[:], Cacc[:], lg[:])
                rinv = recp.tile([1, B], f32, tag="rinv")
                nc.vector.reciprocal(rinv[:], ps_s[:])
                ps_bc = psW.tile([128, B], f32, tag="ps_bc")
                nc.tensor.matmul(ps_bc[:], ones_row[:], rinv[:],
                                 start=True, stop=True)
                bc = recp.tile([128, B], f32, tag="bc")
                nc.vector.tensor_copy(bc[:], ps_bc[:])
                nc.vector.tensor_mul(u_new[:], ps_as[:], bc[:])
                for dj in range(1, 6):
                    uj = u_ring[(t - dj + 1) % 7]
                    nc.vector.tensor_mul(uj[:], uj[:], bc[:])
            else:
                nc.vector.tensor_copy(u_new[:], ps_as[:])


def _build():
    import concourse.bass as bass
    import concourse.tile as tile
    from concourse import mybir, masks

    f32 = mybir.dt.float32
    bf16 = mybir.dt.bfloat16
    AF = mybir.ActivationFunctionType

    nc = bass.Bass()
    uT = nc.dram_tensor("uT", [128, I1_TILES, B], bf16, kind="ExternalInput")
    Wi = nc.dram_tensor("Wi", [128, I1_TILES, K], bf16, kind="ExternalInput")
    Wc1 = nc.dram_tensor("Wc1", [64, 128, I1_TILES, 128], bf16,
                         kind="ExternalInput")
    Wc2 = nc.dram_tensor("Wc2", [N_CHUNKS, 128, I2_TILES, CCHUNK], bf16,
                         kind="ExternalInput")
    tsc = nc.dram_tensor("tsc", [K, K], f32, kind="ExternalInput")
    obsT = nc.dram_tensor("obsT", [T, 128, L, B], bf16, kind="ExternalInput")
    out = nc.dram_tensor("out", [1, B], f32, kind="ExternalOutput")

    with tile.TileContext(nc) as tc:
        with (
            tc.tile_pool(name="const", bufs=1) as constp,
            tc.tile_pool(name="uw", bufs=1) as uwp,
            tc.tile_pool(name="small", bufs=2) as smallp,
            tc.tile_pool(name="uring", bufs=1) as uringp,
        ):
            ident_bf = constp.tile([128, 128], bf16, tag="ident_bf")
            masks.make_identity(nc, ident_bf[:])
            ident_f32 = constp.tile([128, 128], f32, tag="ident_f32")
            masks.make_identity(nc, ident_f32[:])
            ones_col = constp.tile([128, 1], bf16, tag="ones_col")
            nc.vector.memset(ones_col[:], 1.0)
            ones_row = constp.tile([1, 128], f32, tag="ones_row")
            nc.vector.memset(ones_row[:], 1.0)
            tsc_sb = constp.tile([K, K], f32, tag="tsc_sb")
            nc.sync.dma_start(tsc_sb[:], tsc[:])
            Cacc = constp.tile([1, B], f32, tag="Cacc")
            nc.vector.memset(Cacc[:], 0.0)
            lenlp_sb = constp.tile([128, 1], f32, tag="lenlp_sb")
            nc.vector.memset(lenlp_sb[:], LEN_LP)

            u_ring = [uringp.tile([K, B], f32, name=f"u{j}", tag=f"u{j}")
                      for j in range(7)]
            for j in range(1, 7):
                nc.vector.memset(u_ring[j][:], 0.0)

            uT_sb = uwp.tile([128, I1_TILES, B], bf16, tag="uT_sb")
            nc.sync.dma_start(uT_sb[:], uT[:])
            Wi_sb = uwp.tile([128, I1_TILES, K], bf16, tag="Wi_sb")
            nc.sync.dma_start(Wi_sb[:], Wi[:])

            _stage0_pi(nc, tc, bass, mybir, AF, uT_sb, Wi_sb, smallp,
                       ident_f32, u_ring)

            with tc.tile_pool(name="hT", bufs=1) as hTp:
                hT_sb = hTp.tile([128, I2_TILES, B], bf16, tag="hT_sb")
                _stage1_hT(nc, tc, bass, mybir, AF, Wc1, uT_sb, hT_sb)

                with tc.tile_pool(name="condT", bufs=1) as condp:
                    cfT_sb = condp.tile([A_DIM, K, B], bf16, tag="cfT_sb")
                    ctT_sb = condp.tile([A_DIM, K, B], bf16, tag="ctT_sb")
                    _stage2_condT(nc, tc, bass, mybir, Wc2, hT_sb, cfT_sb,
                                  ctT_sb, ident_bf)

                    with tc.tile_pool(name="P", bufs=1) as Pp:
                        P_sb = Pp.tile([K, B, K], bf16, tag="P_sb")
                        _stage3_P(nc, tc, bass, mybir, AF, cfT_sb, ctT_sb,
                                  tsc_sb, P_sb)
                        _recurrence(nc, tc, bass, mybir, AF, P_sb, obsT,
                                    u_ring, Cacc, ones_col, ones_row,
                                    lenlp_sb, out)

    _split_excess_waits(nc, max_waits=1)
    return nc


def _prep_inputs(uniqenc, obs_lps, W_init, b_init, A_from, A_to, W_c1, b_c1,
                 W_c2, b_c2):
    """Host-side shard/cast/tile. Returns in_maps (list of dicts per core)."""
    uniqenc = np.asarray(uniqenc, np.float32)
    obs_lps = np.asarray(obs_lps, np.float32)

    def fold_bias(W, b):
        Wp = np.zeros((W.shape[0] + 128, W.shape[1]), np.float32)
        Wp[:W.shape[0]] = W
        Wp[W.shape[0]] = b
        return Wp

    Wi_p = fold_bias(np.asarray(W_init, np.float32),
                     np.asarray(b_init, np.float32))          # [1152, 128]
    Wc1_p = fold_bias(np.asarray(W_c1, np.float32),
                      np.asarray(b_c1, np.float32))           # [1152, 8192]
    Wc2_p = fold_bias(np.asarray(W_c2, np.float32),
                      np.asarray(b_c2, np.float32))           # [8320, 16384]

    Wi_t = np.ascontiguousarray(
        Wi_p.reshape(I1_TILES, 128, K).transpose(1, 0, 2)).astype(BF16)
    Wc1_t = np.ascontiguousarray(
        Wc1_p.reshape(I1_TILES, 128, 64, 128).transpose(2, 1, 0, 3)).astype(BF16)
    Wc2_t = np.ascontiguousarray(
        Wc2_p.reshape(I2_TILES, 128, N_CHUNKS, CCHUNK)
        .transpose(2, 1, 0, 3)).astype(BF16)

    tsc = (np.asarray(A_from, np.float32) @ np.asarray(A_to, np.float32))
    np.fill_diagonal(tsc, NEGINF)
    tsc = np.ascontiguousarray(tsc, dtype=np.float32)

    in_maps = []
    for c in range(N_CORES):
        sl = slice(c * B, (c + 1) * B)
        uT_c = np.zeros((128, I1_TILES, B), np.float32)
        ub = uniqenc[sl].T.reshape(8, 128, B)                 # [g, p, b]
        uT_c[:, :8, :] = ub.transpose(1, 0, 2)
        uT_c[0, 8, :] = 1.0                                   # bias row 1024
        obs_c = np.zeros((T, 128, L, B), np.float32)
        for l in range(L):
            # obs_c[t, k, l, b] = obs_lps[l, t-l, b, k]
            obs_c[l:, :, l, :] = obs_lps[l, :T - l, sl, :].transpose(0, 2, 1)
        in_maps.append({
            "uT": uT_c.astype(BF16),
            "Wi": Wi_t,
            "Wc1": Wc1_t,
            "Wc2": Wc2_t,
            "tsc": tsc,
            "obsT": obs_c.astype(BF16),
        })
    return in_maps


def _get_nc():
    if "nc" not in _CACHE:
        _CACHE["nc"] = _build()
    return _CACHE["nc"]


def kernel(uniqenc, obs_lps, W_init, b_init, A_from, A_to, W_c1, b_c1,
           W_c2, b_c2):
    from concourse.bass_utils import run_bass_kernel_spmd

    nc = _get_nc()
    in_maps = _prep_inputs(uniqenc, obs_lps, W_init, b_init, A_from, A_to,
                           W_c1, b_c1, W_c2, b_c2)
    res = run_bass_kernel_spmd(nc, in_maps, core_ids=list(range(N_CORES)))
    outs = [np.asarray(res.results[c]["out"], np.float32).reshape(B)
            for c in range(N_CORES)]
    return np.concatenate(outs, axis=0)
